# revision 16
# baseline (speedup 1.0000x reference)
"""GQA self-attention kernel for Trainium2, sharded over 8 NeuronCores.

Problem: x[4, 2048, 1024], 16 heads / 4 KV groups / head_dim 64.
Sharding: batch (4) x head-half (2 KV groups each) = 8 cores.

All matmuls run in bf16 (PE 1 cycle/col; fp32 runs multi-pass; K=64
matmuls run at HALF rate, so scores contract over K=128 with the unused 64
q rows zeroed). exp runs on ACT over 1024-wide tiles spanning two PSUM
banks. The kernel is software-pipelined: kv projections are a short
prefix; each attention chunk qc interleaves the NEXT chunk's q projection
and the PREVIOUS chunk's out-projection into its ACT-bound loop.

Per-core dataflow (features on partitions):
  xT[1024,2048] -> kT[128,2048], vT[128,2048], qT per head (zero-padded)
  vT --PE transpose--> v_aug[seq,65] tiles (ones row appended)
  scores s[k,q] = kT^T . qT_h  (K=128, zero-padded)
  p = exp(s/8)  (ACT, PSUM f32 -> SBUF bf16, 1024 cols/instr)
  av[65,q] += v_aug^T p  (row 64 = softmax denominator)
  avT_norm = av[0:64] * recip_fast(av[64]) bcast (DVE + GpSimd broadcast)
  yT[e,q] = Wo_p^T . avT_norm  -> DRAM (f32)
Host: y[b] = (yT[2b] + yT[2b+1]).T + bo
"""

import sys
import numpy as np
import ml_dtypes

sys.path.insert(0, "/opt/trn_rl_repo")

from contextlib import ExitStack

import concourse.bass as bass
import concourse.bacc as bacc
import concourse.mybir as mybir
from concourse import tile
from concourse.bass_utils import run_bass_kernel_spmd

F32 = mybir.dt.float32
BF16 = mybir.dt.bfloat16
BF16_NP = ml_dtypes.bfloat16

B, S, E = 4, 2048, 1024
NUM_HEADS, NUM_GROUPS, D = 16, 4, 64
CQ = 512          # q cols per core (8 heads)
CK = 128          # kv cols per core (2 groups)
ET = E // 128     # 8 embed K-tiles
SC = S // 512     # 4 seq chunks of 512
KT = S // 128     # 16 key tiles of 128
QT = CQ // 128    # 4 qT partition tiles
SCALE = 1.0 / np.sqrt(np.float32(D))

_NC_CACHE = {}


def build_nc():
    nc = bacc.Bacc(None, target_bir_lowering=False)

    # DRAM layouts are partition-major ([128, blocks, cols]) so each logical
    # load/store is a single dma_start.
    xT = nc.dram_tensor("xT", [128, ET, S], BF16, kind="ExternalInput")
    wq = nc.dram_tensor("wq", [128, ET, CQ], BF16, kind="ExternalInput")
    wk = nc.dram_tensor("wk", [128, ET, CK], BF16, kind="ExternalInput")
    wv = nc.dram_tensor("wv", [128, ET, CK], BF16, kind="ExternalInput")
    wo = nc.dram_tensor("wo", [128, QT, E], BF16, kind="ExternalInput")
    bqd = nc.dram_tensor("bqd", [128, QT], F32, kind="ExternalInput")
    bkd = nc.dram_tensor("bkd", [128, 1], F32, kind="ExternalInput")
    bvd = nc.dram_tensor("bvd", [128, 1], F32, kind="ExternalInput")
    identd = nc.dram_tensor("identd", [128, 128], BF16, kind="ExternalInput")
    onesd = nc.dram_tensor("onesd", [128, 2 * KT], BF16, kind="ExternalInput")
    yT = nc.dram_tensor("yT", [128, ET, S], F32, kind="ExternalOutput")

    with tile.TileContext(nc) as tc, ExitStack() as ctx, \
            nc.allow_low_precision(reason="bf16 matmuls; rel-err budget 2e-2"):
        const = ctx.enter_context(tc.tile_pool(name="const", bufs=1))
        wpool = ctx.enter_context(tc.tile_pool(name="wpool", bufs=1))
        big = ctx.enter_context(tc.tile_pool(name="big", bufs=1))
        ppool = ctx.enter_context(tc.tile_pool(name="ppool", bufs=4))
        avpool = ctx.enter_context(tc.tile_pool(name="avpool", bufs=2))
        npool = ctx.enter_context(tc.tile_pool(name="npool", bufs=3))
        ypool = ctx.enter_context(tc.tile_pool(name="ypool", bufs=3))
        psA = ctx.enter_context(tc.tile_pool(name="psA", bufs=2, space="PSUM"))
        psAV = ctx.enter_context(tc.tile_pool(name="psAV", bufs=2, space="PSUM"))
        psY = ctx.enter_context(tc.tile_pool(name="psY", bufs=2, space="PSUM"))

        # ---- constants ----
        ident = const.tile([128, 128], BF16)
        nc.scalar.dma_start(out=ident[:], in_=identd[:, :])

        # ---- weights + x (x chunk 0 and wk first so kv proj starts early) ----
        issuers = [nc.sync, nc.scalar, nc.gpsimd]
        xt_all = big.tile([128, SC, ET, 512], BF16)  # resident x (32KB)
        wk_sb = wpool.tile([128, ET, CK], BF16)
        nc.gpsimd.dma_start(out=wk_sb[:], in_=wk[:, :, :])
        for et in range(ET):
            issuers[et % 3].dma_start(
                out=xt_all[:, 0, et, :], in_=xT[:, et, 0:512])
        wq_sb = wpool.tile([128, ET, CQ], BF16)
        nc.sync.dma_start(out=wq_sb[:], in_=wq[:, :, :])
        wv_sb = wpool.tile([128, ET, CK], BF16)
        nc.gpsimd.dma_start(out=wv_sb[:], in_=wv[:, :, :])
        for sc in range(1, SC):
            lo = sc * 512
            for et in range(ET):
                issuers[(sc * ET + et) % 3].dma_start(
                    out=xt_all[:, sc, et, :], in_=xT[:, et, lo:lo + 512])
        wo_sb = wpool.tile([128, QT, E], BF16)
        nc.gpsimd.dma_start(out=wo_sb[:], in_=wo[:, :, :])
        bq_sb = wpool.tile([128, QT], F32)
        nc.scalar.dma_start(out=bq_sb[:], in_=bqd[:, :])
        bk_sb = wpool.tile([128, 1], F32)
        nc.scalar.dma_start(out=bk_sb[:], in_=bkd[:, :])
        bv_sb = wpool.tile([128, 1], F32)
        nc.scalar.dma_start(out=bv_sb[:], in_=bvd[:, :])

        # ---- persistent activations ----
        # one head per qT tile; complementary 64 rows stay zero so scores can
        # contract over K=128 (K=64 matmuls run at half rate on TRN2)
        qT_sb = big.tile([128, 8, S], BF16)       # 32KB/partition
        kT_sb = big.tile([128, S], BF16)          # 4KB
        vT_sb = big.tile([128, S], BF16)          # 4KB
        # v natural + ones row, [part, 65, tile] (tile idx = g*KT+kt)
        vaug = big.tile([128, 65, 2 * KT], BF16)
        nc.scalar.dma_start(out=vaug[:, 64, :], in_=onesd[:, :])
        nc.vector.memset(qT_sb[64:128, 0:4, :], 0.0)
        nc.vector.memset(qT_sb[0:64, 4:8, :], 0.0)

        ADD = mybir.AluOpType.add
        MUL = mybir.AluOpType.mult

        def qt_proj_ops(qc, t):
            """Thunks projecting q tile t (heads t and t+4) for chunk qc."""
            lo = qc * 512
            state = {}

            def mk(et):
                def _f():
                    if et == 0:
                        state["pq"] = psAV.tile([128, 512], F32, tag="av",
                                               name=f"pq{qc}_{t}")
                    nc.tensor.matmul(
                        state["pq"][:], wq_sb[:, et, t * 128:(t + 1) * 128],
                        xt_all[:, qc, et, :], start=(et == 0), stop=(et == ET - 1))
                return _f

            def fin():
                pq = state["pq"]
                nc.vector.tensor_scalar(
                    out=qT_sb[0:64, t, lo:lo + 512], in0=pq[0:64, :],
                    scalar1=bq_sb[0:64, t:t + 1], scalar2=None, op0=ADD)
                nc.vector.tensor_scalar(
                    out=qT_sb[64:128, t + 4, lo:lo + 512], in0=pq[64:128, :],
                    scalar1=bq_sb[64:128, t:t + 1], scalar2=None, op0=ADD)
            return [(True, mk(et)) for et in range(ET)] + [(False, fin)]

        def qt_proj(qc, t):
            for _, f in qt_proj_ops(qc, t):
                f()

        def out_proj_ops(qc, et, avT):
            """Thunks for one out-projection column block of chunk qc."""
            lo = qc * 512
            state = {}

            def mk(t):
                def _f():
                    if t == 0:
                        state["yp"] = psY.tile([128, 512], F32, tag="y",
                                               name=f"yp{qc}_{et}")
                    nc.tensor.matmul(
                        state["yp"][:], wo_sb[:, t, et * 128:(et + 1) * 128],
                        avT[:, t, :], start=(t == 0), stop=(t == QT - 1))
                return _f

            def fin():
                ysb = ypool.tile([128, 512], F32, tag="ysb", name=f"ysb{qc}_{et}")
                nc.vector.tensor_copy(out=ysb[:], in_=state["yp"][:])
                nc.sync.dma_start(out=yT[:, et, lo:lo + 512], in_=ysb[:])
            return [(True, mk(t)) for t in range(QT)] + [(False, fin)]

        def out_proj(qc, et, avT):
            for _, f in out_proj_ops(qc, et, avT):
                f()

        # ---- kv projections (prefix) ----
        for sc in range(SC):
            lo = sc * 512
            pk = psA.tile([128, 1024], F32, tag="mm", name=f"pk{sc}")
            for et in range(ET):
                nc.tensor.matmul(pk[:, 0:512], wk_sb[:, et, :],
                                 xt_all[:, sc, et, :],
                                 start=(et == 0), stop=(et == ET - 1))
            nc.vector.tensor_scalar(out=kT_sb[:, lo:lo + 512], in0=pk[:, 0:512],
                                    scalar1=bk_sb[:, 0:1], scalar2=None, op0=ADD)
            pv = psA.tile([128, 1024], F32, tag="mm", name=f"pv{sc}")
            for et in range(ET):
                nc.tensor.matmul(pv[:, 0:512], wv_sb[:, et, :],
                                 xt_all[:, sc, et, :],
                                 start=(et == 0), stop=(et == ET - 1))
            nc.vector.tensor_scalar(out=vT_sb[:, lo:lo + 512], in0=pv[:, 0:512],
                                    scalar1=bv_sb[:, 0:1], scalar2=None, op0=ADD)
            for ktl in range(4):
                kt = sc * 4 + ktl
                ptr = psAV.tile([128, 128], BF16, tag="av", name=f"ptr{kt}")
                nc.tensor.transpose(ptr[:], vT_sb[:, kt * 128:(kt + 1) * 128], ident[:])
                for g in range(2):
                    nc.vector.tensor_copy(
                        out=vaug[:, 0:64, g * KT + kt], in_=ptr[:, g * 64:(g + 1) * 64])

        # q projection: chunk 0 in the prefix; chunks 1-3 trickle through
        # the attention filler slots, one full tile per head so each pq
        # accumulation opens and closes within a single head
        for t in range(QT):
            qt_proj(0, t)

        def head_fillers(qc, h, avT_tiles):
            fills = []
            if qc == 0:
                tgt, t = (1, h) if h < 4 else (2, h - 4)
                fills.extend(qt_proj_ops(tgt, t))
            elif qc == 1:
                if h < 4:
                    fills.extend(qt_proj_ops(3, h))
                else:
                    fills.extend(out_proj_ops(0, 2 * (h - 4), avT_tiles[0]))
                    fills.extend(out_proj_ops(0, 2 * (h - 4) + 1, avT_tiles[0]))
            else:
                fills.extend(out_proj_ops(qc - 1, h, avT_tiles[qc - 1]))
            return fills

        # ---- attention, software-pipelined across chunks ----
        # Flattened (head, kp) pipeline per chunk: scores+exp for pair i+1
        # are emitted BEFORE the AV matmuls of pair i, so the in-order PE
        # always has independent work while ACT computes exp(i), and AV(i)
        # never waits on exp(i) at execution time.
        avT_tiles = {}
        for qc in range(SC):
            lo = qc * 512
            avT = avpool.tile([128, QT, 512], BF16, tag="avT", name=f"avT{qc}")
            avT_tiles[qc] = avT
            NP_ = KT // 2
            pairs = [(h, kp) for h in range(8) for kp in range(NP_)]
            pes = {}

            def scores_exp(h, kp):
                sp = psA.tile([128, 1024], F32, tag="mm", name=f"sp{qc}_{h}_{kp}")
                pe = ppool.tile([128, 1024], BF16, tag="pexp",
                                name=f"pe{qc}_{h}_{kp}")
                for j in range(2):
                    kt = 2 * kp + j
                    nc.tensor.matmul(
                        sp[:, j * 512:(j + 1) * 512],
                        kT_sb[:, kt * 128:(kt + 1) * 128],
                        qT_sb[:, h, lo:lo + 512],
                        start=True, stop=True)
                nc.scalar.activation(
                    pe[:], sp[:], mybir.ActivationFunctionType.Exp,
                    scale=float(SCALE))
                pes[(h, kp)] = pe

            pend = []
            avp = None
            scores_exp(*pairs[0])
            for i, (h, kp) in enumerate(pairs):
                t, g = h % 4, h // 4
                ph = g * 64
                if kp == 0:
                    # new head: queue its fillers
                    pend.extend(head_fillers(qc, h, avT_tiles))
                    avp = psAV.tile([128, 512], F32, tag="av",
                                    name=f"avp{qc}_{h}")
                if i + 1 < len(pairs):
                    scores_exp(*pairs[i + 1])
                pe = pes.pop((h, kp))
                for j in range(2):
                    kt = 2 * kp + j
                    nc.tensor.matmul(
                        avp[0:65, :], vaug[:, :, g * KT + kt],
                        pe[:, j * 512:(j + 1) * 512],
                        start=(kt == 0), stop=(kt == KT - 1))
                emitted_mm = False
                while pend and not emitted_mm:
                    is_mm, f = pend.pop(0)
                    f()
                    emitted_mm = is_mm
                if kp == NP_ - 1:
                    while pend:
                        pend.pop(0)[1]()
                    # normalize: avT_norm = av[0:64] * (1 / av[64])
                    den = npool.tile([1, 512], F32, tag="den",
                                     name=f"den{qc}_{h}")
                    nc.vector.tensor_copy(out=den[:], in_=avp[64:65, :])
                    linv = npool.tile([1, 512], F32, tag="linv",
                                      name=f"linv{qc}_{h}")
                    nc.vector.reciprocal_approx_fast(out=linv[:], in_=den[:])
                    lrep = npool.tile([64, 512], F32, tag="lrep",
                                      name=f"lrep{qc}_{h}")
                    nc.gpsimd.partition_broadcast(out_ap=lrep[:], in_ap=linv[:])
                    nc.vector.tensor_tensor(
                        out=avT[ph:ph + 64, t, :], in0=avp[0:64, :],
                        in1=lrep[:], op=MUL)
        # tail: out-projection for the last chunk
        for et in range(ET):
            out_proj(SC - 1, et, avT_tiles[SC - 1])
    nc.compile()
    return nc


def _pm(a):
    """[E(=n*128), cols] -> partition-major [128, n, cols]."""
    a = np.ascontiguousarray(a)
    n = a.shape[0] // 128
    return np.ascontiguousarray(
        a.reshape(n, 128, a.shape[1]).transpose(1, 0, 2))


def _shard_inputs(x, Wq, bq, Wk, bk, Wv, bv, Wo, bo):
    """Build the 8 per-core input maps."""
    x = np.asarray(x, dtype=np.float32)
    in_maps = []
    for c in range(8):
        b, H = c // 2, c % 2
        heads = [8 * H + t for t in range(4)] + [8 * H + t + 4 for t in range(4)]
        # qT tile t holds (local head t -> partitions 0-63, local head t+4 -> 64-127)
        order = []
        for t in range(4):
            order.extend(range(heads[t] * 64, heads[t] * 64 + 64))
            order.extend(range(heads[t + 4] * 64, heads[t + 4] * 64 + 64))
        order = np.asarray(order)
        wq_p = _pm(np.asarray(Wq, np.float32)[:, order]).astype(BF16_NP)
        bq_p = np.ascontiguousarray(
            np.asarray(bq, np.float32)[order].reshape(4, 128).T)
        wo_p = _pm(np.asarray(Wo, np.float32)[order, :]).astype(BF16_NP)
        wk_s = _pm(np.asarray(Wk, np.float32)[:, H * 128:(H + 1) * 128]).astype(BF16_NP)
        wv_s = _pm(np.asarray(Wv, np.float32)[:, H * 128:(H + 1) * 128]).astype(BF16_NP)
        bk_s = np.ascontiguousarray(np.asarray(bk, np.float32)[H * 128:(H + 1) * 128]
                                    .reshape(128, 1))
        bv_s = np.ascontiguousarray(np.asarray(bv, np.float32)[H * 128:(H + 1) * 128]
                                    .reshape(128, 1))
        xT_b = _pm(np.ascontiguousarray(x[b].T)).astype(BF16_NP)
        in_maps.append({
            "xT": xT_b, "wq": wq_p, "wk": wk_s, "wv": wv_s, "wo": wo_p,
            "bqd": bq_p, "bkd": bk_s, "bvd": bv_s,
            "identd": np.eye(128, dtype=BF16_NP),
            "onesd": np.ones((128, 2 * KT), dtype=BF16_NP),
        })
    return in_maps


def kernel(x, Wq, bq, Wk, bk, Wv, bv, Wo, bo, _trace=False):
    if "nc" not in _NC_CACHE:
        _NC_CACHE["nc"] = build_nc()
    nc = _NC_CACHE["nc"]
    in_maps = _shard_inputs(x, Wq, bq, Wk, bk, Wv, bv, Wo, bo)
    res = run_bass_kernel_spmd(nc, in_maps, list(range(8)), trace=_trace)
    bo = np.asarray(bo, dtype=np.float32)
    out = np.empty((B, S, E), dtype=np.float32)
    for b in range(B):
        # yT dram layout [128, ET, S] -> [E, S]
        yTa = res.results[2 * b]["yT"] + res.results[2 * b + 1]["yT"]
        yE = yTa.transpose(1, 0, 2).reshape(E, S)
        out[b] = yE.T + bo
    if _trace:
        return out, res
    return out


# revision 17
# speedup vs baseline: 1.0665x; 1.0665x over previous
"""GQA self-attention kernel for Trainium2, sharded over 8 NeuronCores.

Problem: x[4, 2048, 1024], 16 heads / 4 KV groups / head_dim 64.
Sharding: batch (4) x head-half (2 KV groups each) = 8 cores.

All matmuls run in bf16 (PE 1 cycle/col; fp32 runs multi-pass; K=64
matmuls run at HALF rate, so scores contract over K=128 with the unused 64
q rows zeroed). exp runs on ACT over 1024-wide tiles spanning two PSUM
banks. The kernel is software-pipelined: kv projections are a short
prefix; each attention chunk qc interleaves the NEXT chunk's q projection
and the PREVIOUS chunk's out-projection into its ACT-bound loop.

Per-core dataflow (features on partitions):
  xT[1024,2048] -> kT[128,2048], vT[128,2048], qT per head (zero-padded)
  vT --PE transpose--> v_aug[seq,65] tiles (ones row appended)
  scores s[k,q] = kT^T . qT_h  (K=128, zero-padded)
  p = exp(s/8)  (ACT, PSUM f32 -> SBUF bf16, 1024 cols/instr)
  av[65,q] += v_aug^T p  (row 64 = softmax denominator)
  avT_norm = av[0:64] * recip_fast(av[64]) bcast (DVE + GpSimd broadcast)
  yT[e,q] = Wo_p^T . avT_norm  -> DRAM (f32)
Host: y[b] = (yT[2b] + yT[2b+1]).T + bo
"""

import sys
import numpy as np
import ml_dtypes

sys.path.insert(0, "/opt/trn_rl_repo")

from contextlib import ExitStack

import concourse.bass as bass
import concourse.bacc as bacc
import concourse.mybir as mybir
from concourse import tile
from concourse.bass_utils import run_bass_kernel_spmd

F32 = mybir.dt.float32
BF16 = mybir.dt.bfloat16
BF16_NP = ml_dtypes.bfloat16

B, S, E = 4, 2048, 1024
NUM_HEADS, NUM_GROUPS, D = 16, 4, 64
CQ = 512          # q cols per core (8 heads)
CK = 128          # kv cols per core (2 groups)
ET = E // 128     # 8 embed K-tiles
SC = S // 512     # 4 seq chunks of 512
KT = S // 128     # 16 key tiles of 128
QT = CQ // 128    # 4 qT partition tiles
SCALE = 1.0 / np.sqrt(np.float32(D))

_NC_CACHE = {}


def build_nc():
    nc = bacc.Bacc(None, target_bir_lowering=False)

    # DRAM layouts are partition-major ([128, blocks, cols]) so each logical
    # load/store is a single dma_start.
    xT = nc.dram_tensor("xT", [128, ET, S], BF16, kind="ExternalInput")
    wq = nc.dram_tensor("wq", [128, ET, CQ], BF16, kind="ExternalInput")
    wk = nc.dram_tensor("wk", [128, ET, CK], BF16, kind="ExternalInput")
    wv = nc.dram_tensor("wv", [128, ET, CK], BF16, kind="ExternalInput")
    wo = nc.dram_tensor("wo", [128, QT, E], BF16, kind="ExternalInput")
    bqd = nc.dram_tensor("bqd", [128, QT], F32, kind="ExternalInput")
    bkd = nc.dram_tensor("bkd", [128, 1], F32, kind="ExternalInput")
    bvd = nc.dram_tensor("bvd", [128, 1], F32, kind="ExternalInput")
    identd = nc.dram_tensor("identd", [128, 128], BF16, kind="ExternalInput")
    onesd = nc.dram_tensor("onesd", [128, 2 * KT], BF16, kind="ExternalInput")
    yT = nc.dram_tensor("yT", [128, ET, S], F32, kind="ExternalOutput")

    with tile.TileContext(nc) as tc, ExitStack() as ctx, \
            nc.allow_low_precision(reason="bf16 matmuls; rel-err budget 2e-2"):
        const = ctx.enter_context(tc.tile_pool(name="const", bufs=1))
        wpool = ctx.enter_context(tc.tile_pool(name="wpool", bufs=1))
        big = ctx.enter_context(tc.tile_pool(name="big", bufs=1))
        ppool = ctx.enter_context(tc.tile_pool(name="ppool", bufs=4))
        avpool = ctx.enter_context(tc.tile_pool(name="avpool", bufs=2))
        npool = ctx.enter_context(tc.tile_pool(name="npool", bufs=3))
        ypool = ctx.enter_context(tc.tile_pool(name="ypool", bufs=3))
        psA = ctx.enter_context(tc.tile_pool(name="psA", bufs=2, space="PSUM"))
        psAV = ctx.enter_context(tc.tile_pool(name="psAV", bufs=2, space="PSUM"))
        psY = ctx.enter_context(tc.tile_pool(name="psY", bufs=2, space="PSUM"))

        # ---- constants ----
        ident = const.tile([128, 128], BF16)
        nc.scalar.dma_start(out=ident[:], in_=identd[:, :])

        # ---- weights + x (x chunk 0 and wk first so kv proj starts early) ----
        issuers = [nc.sync, nc.scalar, nc.gpsimd]
        xt_all = big.tile([128, SC, ET, 512], BF16)  # resident x (32KB)
        wk_sb = wpool.tile([128, ET, CK], BF16)
        nc.gpsimd.dma_start(out=wk_sb[:], in_=wk[:, :, :])
        for et in range(ET):
            issuers[et % 3].dma_start(
                out=xt_all[:, 0, et, :], in_=xT[:, et, 0:512])
        wq_sb = wpool.tile([128, ET, CQ], BF16)
        nc.sync.dma_start(out=wq_sb[:], in_=wq[:, :, :])
        wv_sb = wpool.tile([128, ET, CK], BF16)
        nc.gpsimd.dma_start(out=wv_sb[:], in_=wv[:, :, :])
        for sc in range(1, SC):
            lo = sc * 512
            for et in range(ET):
                issuers[(sc * ET + et) % 3].dma_start(
                    out=xt_all[:, sc, et, :], in_=xT[:, et, lo:lo + 512])
        wo_sb = wpool.tile([128, QT, E], BF16)
        nc.gpsimd.dma_start(out=wo_sb[:], in_=wo[:, :, :])
        bq_sb = wpool.tile([128, QT], F32)
        nc.scalar.dma_start(out=bq_sb[:], in_=bqd[:, :])
        bk_sb = wpool.tile([128, 1], F32)
        nc.scalar.dma_start(out=bk_sb[:], in_=bkd[:, :])
        bv_sb = wpool.tile([128, 1], F32)
        nc.scalar.dma_start(out=bv_sb[:], in_=bvd[:, :])

        # ---- persistent activations ----
        # one head per qT tile; complementary 64 rows stay zero so scores can
        # contract over K=128 (K=64 matmuls run at half rate on TRN2)
        qT_sb = big.tile([128, 8, S], BF16)       # 32KB/partition
        kT_sb = big.tile([128, S], BF16)          # 4KB
        vT_sb = big.tile([128, S], BF16)          # 4KB
        # v natural + ones row, [part, 65, tile] (tile idx = g*KT+kt)
        vaug = big.tile([128, 65, 2 * KT], BF16)
        nc.scalar.dma_start(out=vaug[:, 64, :], in_=onesd[:, :])
        nc.vector.memset(qT_sb[64:128, 0:4, :], 0.0)
        nc.vector.memset(qT_sb[0:64, 4:8, :], 0.0)

        ADD = mybir.AluOpType.add
        MUL = mybir.AluOpType.mult

        def qt_proj_ops(qc, t):
            """Thunks projecting q tile t (heads t and t+4) for chunk qc."""
            lo = qc * 512
            state = {}

            def mk(et):
                def _f():
                    if et == 0:
                        state["pq"] = psY.tile([128, 512], F32, tag="y",
                                               name=f"pq{qc}_{t}")
                    nc.tensor.matmul(
                        state["pq"][:], wq_sb[:, et, t * 128:(t + 1) * 128],
                        xt_all[:, qc, et, :], start=(et == 0), stop=(et == ET - 1))
                return _f

            def fin():
                pq = state["pq"]
                nc.vector.tensor_scalar(
                    out=qT_sb[0:64, t, lo:lo + 512], in0=pq[0:64, :],
                    scalar1=bq_sb[0:64, t:t + 1], scalar2=None, op0=ADD)
                nc.vector.tensor_scalar(
                    out=qT_sb[64:128, t + 4, lo:lo + 512], in0=pq[64:128, :],
                    scalar1=bq_sb[64:128, t:t + 1], scalar2=None, op0=ADD)
            return [(True, mk(et)) for et in range(ET)] + [(False, fin)]

        def qt_proj(qc, t):
            for _, f in qt_proj_ops(qc, t):
                f()

        def out_proj_ops(qc, et, avT):
            """Thunks for one out-projection column block of chunk qc."""
            lo = qc * 512
            state = {}

            def mk(t):
                def _f():
                    if t == 0:
                        state["yp"] = psY.tile([128, 512], F32, tag="y",
                                               name=f"yp{qc}_{et}")
                    nc.tensor.matmul(
                        state["yp"][:], wo_sb[:, t, et * 128:(et + 1) * 128],
                        avT[:, t, :], start=(t == 0), stop=(t == QT - 1))
                return _f

            def fin():
                ysb = ypool.tile([128, 512], F32, tag="ysb", name=f"ysb{qc}_{et}")
                nc.vector.tensor_copy(out=ysb[:], in_=state["yp"][:])
                nc.sync.dma_start(out=yT[:, et, lo:lo + 512], in_=ysb[:])
            return [(True, mk(t)) for t in range(QT)] + [(False, fin)]

        def out_proj(qc, et, avT):
            for _, f in out_proj_ops(qc, et, avT):
                f()

        # ---- kv projections (prefix) ----
        for sc in range(SC):
            lo = sc * 512
            pk = psA.tile([128, 1024], F32, tag="mm", name=f"pk{sc}")
            for et in range(ET):
                nc.tensor.matmul(pk[:, 0:512], wk_sb[:, et, :],
                                 xt_all[:, sc, et, :],
                                 start=(et == 0), stop=(et == ET - 1))
            nc.vector.tensor_scalar(out=kT_sb[:, lo:lo + 512], in0=pk[:, 0:512],
                                    scalar1=bk_sb[:, 0:1], scalar2=None, op0=ADD)
            pv = psA.tile([128, 1024], F32, tag="mm", name=f"pv{sc}")
            for et in range(ET):
                nc.tensor.matmul(pv[:, 0:512], wv_sb[:, et, :],
                                 xt_all[:, sc, et, :],
                                 start=(et == 0), stop=(et == ET - 1))
            nc.vector.tensor_scalar(out=vT_sb[:, lo:lo + 512], in0=pv[:, 0:512],
                                    scalar1=bv_sb[:, 0:1], scalar2=None, op0=ADD)
            for ktl in range(4):
                kt = sc * 4 + ktl
                ptr = psAV.tile([128, 128], BF16, tag="av", name=f"ptr{kt}")
                nc.tensor.transpose(ptr[:], vT_sb[:, kt * 128:(kt + 1) * 128], ident[:])
                for g in range(2):
                    nc.vector.tensor_copy(
                        out=vaug[:, 0:64, g * KT + kt], in_=ptr[:, g * 64:(g + 1) * 64])

        # q projection: chunk 0 in the prefix; chunks 1-3 trickle through
        # the attention filler slots, one full tile per head so each pq
        # accumulation opens and closes within a single head
        for t in range(QT):
            qt_proj(0, t)

        def head_fillers(qc, h, avT_tiles):
            fills = []
            if qc == 0:
                tgt, t = (1, h) if h < 4 else (2, h - 4)
                fills.extend(qt_proj_ops(tgt, t))
            elif qc == 1:
                if h < 4:
                    fills.extend(qt_proj_ops(3, h))
                else:
                    fills.extend(out_proj_ops(0, 2 * (h - 4), avT_tiles[0]))
                    fills.extend(out_proj_ops(0, 2 * (h - 4) + 1, avT_tiles[0]))
            else:
                fills.extend(out_proj_ops(qc - 1, h, avT_tiles[qc - 1]))
            return fills

        # ---- attention, software-pipelined across chunks ----
        # Flattened (head, kp) pipeline per chunk: scores+exp for pair i+1
        # are emitted BEFORE the AV matmuls of pair i, so the in-order PE
        # always has independent work while ACT computes exp(i), and AV(i)
        # never waits on exp(i) at execution time.
        avT_tiles = {}
        for qc in range(SC):
            lo = qc * 512
            avT = avpool.tile([128, QT, 512], BF16, tag="avT", name=f"avT{qc}")
            avT_tiles[qc] = avT
            NP_ = KT // 2
            pairs = [(h, kp) for h in range(8) for kp in range(NP_)]
            pes = {}

            def scores_exp(h, kp):
                sp = psA.tile([128, 1024], F32, tag="mm", name=f"sp{qc}_{h}_{kp}")
                pe = ppool.tile([128, 1024], BF16, tag="pexp",
                                name=f"pe{qc}_{h}_{kp}")
                for j in range(2):
                    kt = 2 * kp + j
                    nc.tensor.matmul(
                        sp[:, j * 512:(j + 1) * 512],
                        kT_sb[:, kt * 128:(kt + 1) * 128],
                        qT_sb[:, h, lo:lo + 512],
                        start=True, stop=True)
                nc.scalar.activation(
                    pe[:], sp[:], mybir.ActivationFunctionType.Exp,
                    scale=float(SCALE))
                pes[(h, kp)] = pe

            pend = []
            avp = None
            scores_exp(*pairs[0])
            for i, (h, kp) in enumerate(pairs):
                t, g = h % 4, h // 4
                ph = g * 64
                if kp == 0:
                    # new head: queue its fillers
                    pend.extend(head_fillers(qc, h, avT_tiles))
                    avp = psAV.tile([128, 512], F32, tag="av",
                                    name=f"avp{qc}_{h}")
                if i + 1 < len(pairs):
                    scores_exp(*pairs[i + 1])
                pe = pes.pop((h, kp))
                for j in range(2):
                    kt = 2 * kp + j
                    nc.tensor.matmul(
                        avp[0:65, :], vaug[:, :, g * KT + kt],
                        pe[:, j * 512:(j + 1) * 512],
                        start=(kt == 0), stop=(kt == KT - 1))
                emitted_mm = False
                while pend and not emitted_mm:
                    is_mm, f = pend.pop(0)
                    f()
                    emitted_mm = is_mm
                if kp == NP_ - 1:
                    while pend:
                        pend.pop(0)[1]()
                    # normalize: avT_norm = av[0:64] * (1 / av[64])
                    den = npool.tile([1, 512], F32, tag="den",
                                     name=f"den{qc}_{h}")
                    nc.vector.tensor_copy(out=den[:], in_=avp[64:65, :])
                    linv = npool.tile([1, 512], F32, tag="linv",
                                      name=f"linv{qc}_{h}")
                    nc.vector.reciprocal_approx_fast(out=linv[:], in_=den[:])
                    lrep = npool.tile([64, 512], F32, tag="lrep",
                                      name=f"lrep{qc}_{h}")
                    nc.gpsimd.partition_broadcast(out_ap=lrep[:], in_ap=linv[:])
                    nc.vector.tensor_tensor(
                        out=avT[ph:ph + 64, t, :], in0=avp[0:64, :],
                        in1=lrep[:], op=MUL)
        # tail: out-projection for the last chunk
        for et in range(ET):
            out_proj(SC - 1, et, avT_tiles[SC - 1])
    nc.compile()
    return nc


def _pm(a):
    """[E(=n*128), cols] -> partition-major [128, n, cols]."""
    a = np.ascontiguousarray(a)
    n = a.shape[0] // 128
    return np.ascontiguousarray(
        a.reshape(n, 128, a.shape[1]).transpose(1, 0, 2))


def _shard_inputs(x, Wq, bq, Wk, bk, Wv, bv, Wo, bo):
    """Build the 8 per-core input maps."""
    x = np.asarray(x, dtype=np.float32)
    in_maps = []
    for c in range(8):
        b, H = c // 2, c % 2
        heads = [8 * H + t for t in range(4)] + [8 * H + t + 4 for t in range(4)]
        # qT tile t holds (local head t -> partitions 0-63, local head t+4 -> 64-127)
        order = []
        for t in range(4):
            order.extend(range(heads[t] * 64, heads[t] * 64 + 64))
            order.extend(range(heads[t + 4] * 64, heads[t + 4] * 64 + 64))
        order = np.asarray(order)
        wq_p = _pm(np.asarray(Wq, np.float32)[:, order]).astype(BF16_NP)
        bq_p = np.ascontiguousarray(
            np.asarray(bq, np.float32)[order].reshape(4, 128).T)
        wo_p = _pm(np.asarray(Wo, np.float32)[order, :]).astype(BF16_NP)
        wk_s = _pm(np.asarray(Wk, np.float32)[:, H * 128:(H + 1) * 128]).astype(BF16_NP)
        wv_s = _pm(np.asarray(Wv, np.float32)[:, H * 128:(H + 1) * 128]).astype(BF16_NP)
        bk_s = np.ascontiguousarray(np.asarray(bk, np.float32)[H * 128:(H + 1) * 128]
                                    .reshape(128, 1))
        bv_s = np.ascontiguousarray(np.asarray(bv, np.float32)[H * 128:(H + 1) * 128]
                                    .reshape(128, 1))
        xT_b = _pm(np.ascontiguousarray(x[b].T)).astype(BF16_NP)
        in_maps.append({
            "xT": xT_b, "wq": wq_p, "wk": wk_s, "wv": wv_s, "wo": wo_p,
            "bqd": bq_p, "bkd": bk_s, "bvd": bv_s,
            "identd": np.eye(128, dtype=BF16_NP),
            "onesd": np.ones((128, 2 * KT), dtype=BF16_NP),
        })
    return in_maps


def kernel(x, Wq, bq, Wk, bk, Wv, bv, Wo, bo, _trace=False):
    if "nc" not in _NC_CACHE:
        _NC_CACHE["nc"] = build_nc()
    nc = _NC_CACHE["nc"]
    in_maps = _shard_inputs(x, Wq, bq, Wk, bk, Wv, bv, Wo, bo)
    res = run_bass_kernel_spmd(nc, in_maps, list(range(8)), trace=_trace)
    bo = np.asarray(bo, dtype=np.float32)
    out = np.empty((B, S, E), dtype=np.float32)
    for b in range(B):
        # yT dram layout [128, ET, S] -> [E, S]
        yTa = res.results[2 * b]["yT"] + res.results[2 * b + 1]["yT"]
        yE = yTa.transpose(1, 0, 2).reshape(E, S)
        out[b] = yE.T + bo
    if _trace:
        return out, res
    return out
